# revision 12
# baseline (speedup 1.0000x reference)
"""Trainium2 Bass kernel for nn_ContrastiveDist (supervised contrastive loss).

Math
----
The reference builds (n,n) distance/weight matrices, but the loss collapses
exactly to per-class statistics.  With classes c = 0..15, per-class count
cnt[c], feature sums C[c,:], squared-norm sums SqSum[c], global sums
Ftot / SSall:

    alpha[c] = 1/(cnt[c]-1+eps)
    beta[c]  = 1/(n-cnt[c]+eps)
    loss_i   = sq_i*P[c_i] + (Q[c_i]+M) + f_i . R[c_i]
      P[c]   = alpha*cnt - beta*(n-cnt)
      Q[c]   = alpha*SqSum[c] - beta*(SSall-SqSum[c])
      R[c,:] = 2*beta*(Ftot-C[c]) - 2*alpha*C[c]
    result   = sum(relu(loss_i)*valid_i) / max(sum(valid_i), 1)

valid_i = (cnt[c_i] >= 2) is folded into the coefficients: Raug rows of
invalid classes are zeroed, so relu(loss) = 0 there, and the valid count
comes from sum(cnt[c]*vmask[c]).

Precision: feature path is single-chain bf16 with fp32 PSUM accumulation;
numpy pipeline sim puts the result at rel err <= ~6e-4 (harness gate 2e-2).

Perf structure (per core, inputs replicated -- collectives measured at
~45us/call under this dispatch path, so none are used):
  - one bf16 copy of F in faug layout [f(128) | sq | 1 | pad], W=132 so
    every tile slice is 4B-aligned (keeps DVE bf16 2x packing legal)
  - one-hots precomputed on host (no DVE is_equal work)
  - sq: chunked Square (split Scalar/DVE) + DVE reduce, pipelined with load
  - stats: 64 accumulating matmuls chunk-pipelined behind the load
  - loss: per tile-pair, 2 matmuls gather [R|P|QM] into one PSUM bank,
    Scalar copies PSUM->SBUF bf16 (exact: values are bf16), DVE dots run
    all-SBUF-bf16 at the 2x perf mode with accum_out
"""

import numpy as np
import ml_dtypes

import concourse.bacc as bacc
import concourse.tile as tile
import concourse.mybir as mybir
from concourse.bass_utils import run_bass_kernel_spmd

N, D, K, NCORES = 8192, 128, 16, 8
T = N // 128               # 64 row-tiles of 128
W = D + 4                  # faug stride: [F(128), sq, 1, pad, pad]
EPS, MARGIN = 1e-6, 10.0
F32 = mybir.dt.float32
BF16 = mybir.dt.bfloat16
Alu = mybir.AluOpType
Act = mybir.ActivationFunctionType
AxX = mybir.AxisListType.X

# const tensor (128, CW) f32:
#   col 0       1.0 (ones(128,1) lhsT for the final partition reduce)
#   cols 1:17   1.0 in rows 0:16 (ones(16,16) lhsT for global-sum broadcast)
CW = 18

_CACHE: dict = {}


def _build():
    if "nc" in _CACHE:
        return _CACHE["nc"]

    nc = bacc.Bacc("TRN2", target_bir_lowering=False, debug=False, num_devices=NCORES)
    fbin = nc.dram_tensor("fbin", [128, T * W], BF16, kind="ExternalInput").ap()
    eohin = nc.dram_tensor("eohin", [128, T * 16], BF16, kind="ExternalInput").ap()
    eohTin = nc.dram_tensor("eohTin", [16, N], BF16, kind="ExternalInput").ap()
    cst = nc.dram_tensor("cst", [128, CW], F32, kind="ExternalInput").ap()
    res = nc.dram_tensor("res", [1, 1], F32, kind="ExternalOutput").ap()

    with tile.TileContext(nc) as tc:
        with (
            tc.tile_pool(name="sb", bufs=1) as sb,
            tc.tile_pool(name="ps", bufs=1, space="PSUM") as ps,
        ):
            # ---------------- loads ----------------
            # Everything on the two HWDGE rings (sync/scalar) -- no gpsimd
            # instructions anywhere in the kernel, so the Pool sequencer
            # contributes nothing to the epilogue.  Side tensors are slotted
            # where they're needed: eoh early (stats lhsT), eohT/cst late.
            faug = sb.tile([128, T * W], BF16)
            fa3 = faug.rearrange("p (t w) -> p t w", w=W)
            eoh = sb.tile([128, T * 16], BF16)
            eohT = sb.tile([16, N], BF16)
            csts = sb.tile([128, CW], F32)

            # (start_tile, n_tiles, square_engine)
            chunks = [(0, 8, "act"), (8, 12, "dve"), (20, 12, "act"),
                      (32, 12, "dve"), (44, 8, "act"), (52, 6, "dve"),
                      (58, 6, "act")]

            def _chunk_dma(eng, g):
                t0, ntl, _ = chunks[g]
                eng.dma_start(faug[:, t0 * W:(t0 + ntl) * W],
                              fbin[:, t0 * W:(t0 + ntl) * W])

            _chunk_dma(nc.sync, 0)
            nc.sync.dma_start(eoh[:], eohin)
            _chunk_dma(nc.scalar, 1)
            _chunk_dma(nc.sync, 2)
            _chunk_dma(nc.scalar, 3)
            _chunk_dma(nc.sync, 4)
            _chunk_dma(nc.scalar, 5)
            _chunk_dma(nc.sync, 6)
            nc.scalar.dma_start(eohT[:], eohTin)
            nc.sync.dma_start(csts[:], cst)

            eoh3 = eoh.rearrange("p (t c) -> p t c", c=16)

            # ---------- sq + per-class stats, chunk-pipelined with load ----
            sqd = sb.tile([128, T], F32)
            statsP = ps.tile([16, D + 2], F32)
            for g, (t0, ntl, sqeng) in enumerate(chunks):
                scr = sb.tile([128, ntl * D], BF16, tag="sqscr", bufs=2,
                              name=f"scr{g}")
                scr3 = scr.rearrange("p (t d) -> p t d", d=D)
                if sqeng == "act":
                    nc.scalar.activation(scr3[:, :, :],
                                         fa3[:, t0:t0 + ntl, 0:D], Act.Square)
                else:
                    nc.vector.tensor_tensor(scr3[:, :, :],
                                            fa3[:, t0:t0 + ntl, 0:D],
                                            fa3[:, t0:t0 + ntl, 0:D],
                                            op=Alu.mult)
                nc.vector.tensor_reduce(sqd[:, t0:t0 + ntl], scr3,
                                        axis=AxX, op=Alu.add)
                nc.vector.tensor_copy(fa3[:, t0:t0 + ntl, D],
                                      sqd[:, t0:t0 + ntl])
                for j in range(ntl):
                    t = t0 + j
                    nc.tensor.matmul(statsP[:], eoh3[:, t, :],
                                     fa3[:, t, 0:D + 2],
                                     start=(t == 0), stop=(t == T - 1))
            stats = sb.tile([16, D + 2], F32)
            nc.vector.tensor_copy(stats[:], statsP[:])

            # ---------------- per-class coefficients ----------------
            C = stats[:, 0:D]
            SqS = stats[:, D:D + 1]
            cnt = stats[:, D + 1:D + 2]
            gbP = ps.tile([16, D + 2], F32)
            nc.tensor.matmul(gbP[:], csts[0:16, 1:17], stats[:],
                             start=True, stop=True)
            gb = sb.tile([16, D + 2], F32)
            nc.vector.tensor_copy(gb[:], gbP[:])
            Ftot = gb[:, 0:D]
            SSall = gb[:, D:D + 1]

            alpha = sb.tile([16, 1], F32)
            nc.vector.tensor_scalar(alpha[:], cnt, EPS - 1.0, None, op0=Alu.add)
            nc.vector.reciprocal(alpha[:], alpha[:])
            beta = sb.tile([16, 1], F32)
            nc.vector.tensor_scalar(beta[:], cnt, -1.0, float(N) + EPS,
                                    op0=Alu.mult, op1=Alu.add)
            nc.vector.reciprocal(beta[:], beta[:])
            nalpha2 = sb.tile([16, 1], F32)
            nc.vector.tensor_scalar(nalpha2[:], alpha[:], -2.0, None, op0=Alu.mult)
            beta2 = sb.tile([16, 1], F32)
            nc.vector.tensor_scalar(beta2[:], beta[:], 2.0, None, op0=Alu.mult)

            raug = sb.tile([16, D + 2], F32)
            tmpd = sb.tile([16, D], F32)
            nc.vector.tensor_tensor(tmpd[:], Ftot, C, op=Alu.subtract)
            nc.vector.tensor_scalar(tmpd[:], tmpd[:], beta2[:], None, op0=Alu.mult)
            nc.vector.scalar_tensor_tensor(raug[:, 0:D], C, nalpha2[:], tmpd[:],
                                           op0=Alu.mult, op1=Alu.add)
            nmc = sb.tile([16, 1], F32)
            nc.vector.tensor_scalar(nmc[:], cnt, -1.0, float(N),
                                    op0=Alu.mult, op1=Alu.add)
            nc.vector.tensor_tensor(nmc[:], nmc[:], beta[:], op=Alu.mult)
            nc.vector.scalar_tensor_tensor(raug[:, D:D + 1], cnt, alpha[:], nmc[:],
                                           op0=Alu.mult, op1=Alu.subtract)
            ssd = sb.tile([16, 1], F32)
            nc.vector.tensor_tensor(ssd[:], SSall, SqS, op=Alu.subtract)
            nc.vector.tensor_tensor(ssd[:], ssd[:], beta[:], op=Alu.mult)
            qa = sb.tile([16, 1], F32)
            nc.vector.scalar_tensor_tensor(qa[:], SqS, alpha[:], ssd[:],
                                           op0=Alu.mult, op1=Alu.subtract)
            nc.vector.tensor_scalar(raug[:, D + 1:D + 2], qa[:], MARGIN, None,
                                    op0=Alu.add)

            # fold validity into the coefficients: zero Raug rows of classes
            # with cnt < 2, so relu(loss) vanishes for invalid rows
            vmask = sb.tile([16, 1], F32)
            nc.vector.tensor_scalar(vmask[:], cnt, 1.5, None, op0=Alu.is_ge)
            nc.vector.tensor_scalar(raug[:], raug[:], vmask[:], None, op0=Alu.mult)

            rhi = sb.tile([16, D + 2], BF16)
            nc.vector.tensor_copy(rhi[:], raug[:])

            # ---------------- per-row losses ----------------
            lossrows = sb.tile([128, T], F32)
            for t in range(T):
                dP = ps.tile([128, 512], F32, tag="dpsum", bufs=4, name=f"dP{t}")
                nc.tensor.matmul(dP[:, 0:D + 2], eohT[:, t * 128:(t + 1) * 128],
                                 rhi[:], start=True, stop=True)
                pscr = sb.tile([128, D + 2], BF16, tag="pscr", bufs=4,
                               name=f"pt{t}")
                nc.vector.scalar_tensor_tensor(
                    pscr[:], dP[:, 0:D + 2], 0.0, fa3[:, t, 0:D + 2],
                    op0=Alu.bypass, op1=Alu.mult,
                    accum_out=lossrows[:, t:t + 1])

            # ---------------- final reduction ----------------
            accpair = sb.tile([128, 2], F32)
            nc.vector.tensor_copy(accpair[:, 1:2], csts[:, 17:18])  # zeros col
            relscr = sb.tile([128, T], F32)
            nc.vector.tensor_scalar(relscr[:], lossrows[:], 0.0, None,
                                    op0=Alu.max, op1=Alu.add,
                                    accum_out=accpair[:, 0:1])
            nc.vector.tensor_tensor(accpair[0:16, 1:2], cnt, vmask[:],
                                    op=Alu.mult)
            finP = ps.tile([1, 2], F32)
            nc.tensor.matmul(finP[:], csts[:, 0:1], accpair[:],
                             start=True, stop=True)
            fin = sb.tile([1, 2], F32)
            nc.vector.tensor_copy(fin[:], finP[:])
            den = sb.tile([1, 1], F32)
            nc.vector.tensor_scalar(den[:], fin[:, 1:2], 1.0, None, op0=Alu.max)
            nc.vector.reciprocal(den[:], den[:])
            resS = sb.tile([1, 1], F32)
            nc.vector.tensor_tensor(resS[:], fin[:, 0:1], den[:], op=Alu.mult)
            nc.sync.dma_start(res, resS[:])

    nc.compile()
    _CACHE["nc"] = nc
    return nc


def _make_in_maps(features, labels):
    feats = np.ascontiguousarray(np.asarray(features, dtype=np.float32))
    lab = np.ascontiguousarray(np.asarray(labels)).astype(np.int64)

    cst = np.zeros((128, CW), np.float32)
    cst[:, 0] = 1.0
    cst[0:16, 1:17] = 1.0

    fa = np.zeros((128, T, W), np.float32)
    fa[:, :, 0:D] = feats.reshape(T, 128, D).transpose(1, 0, 2)
    fa[:, :, D + 1] = 1.0
    fb = fa.reshape(128, T * W).astype(ml_dtypes.bfloat16)

    labT = lab.reshape(T, 128).T                       # (128, T)
    eoh = (labT[:, :, None] == np.arange(16)[None, None, :])
    eohin = np.ascontiguousarray(
        eoh.reshape(128, T * 16)).astype(ml_dtypes.bfloat16)
    eohT = (lab[None, :] == np.arange(16)[:, None])
    eohTin = np.ascontiguousarray(eohT).astype(ml_dtypes.bfloat16)

    one = {
        "fbin": fb,
        "eohin": eohin,
        "eohTin": eohTin,
        "cst": cst,
    }
    return [dict(one) for _ in range(NCORES)]


def kernel(features, labels):
    nc = _build()
    in_maps = _make_in_maps(features, labels)
    out = run_bass_kernel_spmd(nc, in_maps, core_ids=list(range(NCORES)))
    return np.float32(out.results[0]["res"][0, 0])


# revision 13
# speedup vs baseline: 1.0987x; 1.0987x over previous
"""Trainium2 Bass kernel for nn_ContrastiveDist (supervised contrastive loss).

Math
----
The reference builds (n,n) distance/weight matrices, but the loss collapses
exactly to per-class statistics.  With classes c = 0..15, per-class count
cnt[c], feature sums C[c,:], squared-norm sums SqSum[c], global sums
Ftot / SSall:

    alpha[c] = 1/(cnt[c]-1+eps)
    beta[c]  = 1/(n-cnt[c]+eps)
    loss_i   = sq_i*P[c_i] + (Q[c_i]+M) + f_i . R[c_i]
      P[c]   = alpha*cnt - beta*(n-cnt)
      Q[c]   = alpha*SqSum[c] - beta*(SSall-SqSum[c])
      R[c,:] = 2*beta*(Ftot-C[c]) - 2*alpha*C[c]
    result   = sum(relu(loss_i)*valid_i) / max(sum(valid_i), 1)

valid_i = (cnt[c_i] >= 2) is folded into the coefficients: Raug rows of
invalid classes are zeroed, so relu(loss) = 0 there, and the valid count
comes from sum(cnt[c]*vmask[c]).

Precision: feature path is single-chain bf16 with fp32 PSUM accumulation;
numpy pipeline sim puts the result at rel err <= ~6e-4 (harness gate 2e-2).

Perf structure (per core, inputs replicated -- collectives measured at
~45us/call under this dispatch path, so none are used):
  - one bf16 copy of F in faug layout [f(128) | sq | 1 | pad], W=132 so
    every tile slice is 4B-aligned (keeps DVE bf16 2x packing legal)
  - one-hots precomputed on host (no DVE is_equal work)
  - sq: chunked Square (split Scalar/DVE) + DVE reduce, pipelined with load
  - stats: 64 accumulating matmuls chunk-pipelined behind the load
  - loss: per tile-pair, 2 matmuls gather [R|P|QM] into one PSUM bank,
    Scalar copies PSUM->SBUF bf16 (exact: values are bf16), DVE dots run
    all-SBUF-bf16 at the 2x perf mode with accum_out
"""

import numpy as np
import ml_dtypes

import concourse.bacc as bacc
import concourse.tile as tile
import concourse.mybir as mybir
from concourse.bass_utils import run_bass_kernel_spmd

N, D, K, NCORES = 8192, 128, 16, 8
T = N // 128               # 64 row-tiles of 128
W = D + 4                  # faug stride: [F(128), sq, 1, pad, pad]
EPS, MARGIN = 1e-6, 10.0
F32 = mybir.dt.float32
BF16 = mybir.dt.bfloat16
Alu = mybir.AluOpType
Act = mybir.ActivationFunctionType
AxX = mybir.AxisListType.X

# const tensor (128, CW) f32:
#   col 0       1.0 (ones(128,1) lhsT for the final partition reduce)
#   cols 1:17   1.0 in rows 0:16 (ones(16,16) lhsT for global-sum broadcast)
CW = 18

_CACHE: dict = {}


def _build():
    if "nc" in _CACHE:
        return _CACHE["nc"]

    nc = bacc.Bacc("TRN2", target_bir_lowering=False, debug=False, num_devices=NCORES)
    fbin = nc.dram_tensor("fbin", [128, T * W], BF16, kind="ExternalInput").ap()
    eohin = nc.dram_tensor("eohin", [128, T * 16], BF16, kind="ExternalInput").ap()
    eohTin = nc.dram_tensor("eohTin", [16, N], BF16, kind="ExternalInput").ap()
    cst = nc.dram_tensor("cst", [128, CW], F32, kind="ExternalInput").ap()
    res = nc.dram_tensor("res", [1, 1], F32, kind="ExternalOutput").ap()

    with tile.TileContext(nc) as tc:
        with (
            tc.tile_pool(name="sb", bufs=1) as sb,
            tc.tile_pool(name="ps", bufs=1, space="PSUM") as ps,
        ):
            # ---------------- loads ----------------
            # faug chunks alternate the two HWDGE rings (sync/scalar);
            # side tensors ride the gpsimd SWDGE ring so they never stall
            # the compute-engine sequencers.
            csts = sb.tile([128, CW], F32)
            nc.gpsimd.dma_start(csts[:], cst)
            eoh = sb.tile([128, T * 16], BF16)
            nc.gpsimd.dma_start(eoh[:], eohin)
            eohT = sb.tile([16, N], BF16)
            nc.gpsimd.dma_start(eohT[:], eohTin)

            faug = sb.tile([128, T * W], BF16)
            fa3 = faug.rearrange("p (t w) -> p t w", w=W)
            # (start_tile, n_tiles, square_engine)
            chunks = [(0, 8, "act"), (8, 12, "dve"), (20, 12, "act"),
                      (32, 12, "dve"), (44, 8, "act"), (52, 6, "dve"),
                      (58, 6, "act")]
            qengs = [nc.sync, nc.scalar]
            for g, (t0, ntl, _) in enumerate(chunks):
                eng = qengs[g % len(qengs)]
                eng.dma_start(faug[:, t0 * W:(t0 + ntl) * W],
                              fbin[:, t0 * W:(t0 + ntl) * W])

            eoh3 = eoh.rearrange("p (t c) -> p t c", c=16)

            # ---------- sq + per-class stats, chunk-pipelined with load ----
            sqd = sb.tile([128, T], F32)
            statsP = ps.tile([16, D + 2], F32)
            for g, (t0, ntl, sqeng) in enumerate(chunks):
                scr = sb.tile([128, ntl * D], BF16, tag="sqscr", bufs=2,
                              name=f"scr{g}")
                scr3 = scr.rearrange("p (t d) -> p t d", d=D)
                if sqeng == "act":
                    nc.scalar.activation(scr3[:, :, :],
                                         fa3[:, t0:t0 + ntl, 0:D], Act.Square)
                else:
                    nc.vector.tensor_tensor(scr3[:, :, :],
                                            fa3[:, t0:t0 + ntl, 0:D],
                                            fa3[:, t0:t0 + ntl, 0:D],
                                            op=Alu.mult)
                nc.vector.tensor_reduce(sqd[:, t0:t0 + ntl], scr3,
                                        axis=AxX, op=Alu.add)
                nc.vector.tensor_copy(fa3[:, t0:t0 + ntl, D],
                                      sqd[:, t0:t0 + ntl])
                for j in range(ntl):
                    t = t0 + j
                    nc.tensor.matmul(statsP[:], eoh3[:, t, :],
                                     fa3[:, t, 0:D + 2],
                                     start=(t == 0), stop=(t == T - 1))
            stats = sb.tile([16, D + 2], F32)
            nc.vector.tensor_copy(stats[:], statsP[:])

            # ---------------- per-class coefficients ----------------
            C = stats[:, 0:D]
            SqS = stats[:, D:D + 1]
            cnt = stats[:, D + 1:D + 2]
            gbP = ps.tile([16, D + 2], F32)
            nc.tensor.matmul(gbP[:], csts[0:16, 1:17], stats[:],
                             start=True, stop=True)
            gb = sb.tile([16, D + 2], F32)
            nc.vector.tensor_copy(gb[:], gbP[:])
            Ftot = gb[:, 0:D]
            SSall = gb[:, D:D + 1]

            alpha = sb.tile([16, 1], F32)
            nc.vector.tensor_scalar(alpha[:], cnt, EPS - 1.0, None, op0=Alu.add)
            nc.vector.reciprocal(alpha[:], alpha[:])
            beta = sb.tile([16, 1], F32)
            nc.vector.tensor_scalar(beta[:], cnt, -1.0, float(N) + EPS,
                                    op0=Alu.mult, op1=Alu.add)
            nc.vector.reciprocal(beta[:], beta[:])
            nalpha2 = sb.tile([16, 1], F32)
            nc.vector.tensor_scalar(nalpha2[:], alpha[:], -2.0, None, op0=Alu.mult)
            beta2 = sb.tile([16, 1], F32)
            nc.vector.tensor_scalar(beta2[:], beta[:], 2.0, None, op0=Alu.mult)

            raug = sb.tile([16, D + 2], F32)
            tmpd = sb.tile([16, D], F32)
            nc.vector.tensor_tensor(tmpd[:], Ftot, C, op=Alu.subtract)
            nc.vector.tensor_scalar(tmpd[:], tmpd[:], beta2[:], None, op0=Alu.mult)
            nc.vector.scalar_tensor_tensor(raug[:, 0:D], C, nalpha2[:], tmpd[:],
                                           op0=Alu.mult, op1=Alu.add)
            nmc = sb.tile([16, 1], F32)
            nc.vector.tensor_scalar(nmc[:], cnt, -1.0, float(N),
                                    op0=Alu.mult, op1=Alu.add)
            nc.vector.tensor_tensor(nmc[:], nmc[:], beta[:], op=Alu.mult)
            nc.vector.scalar_tensor_tensor(raug[:, D:D + 1], cnt, alpha[:], nmc[:],
                                           op0=Alu.mult, op1=Alu.subtract)
            ssd = sb.tile([16, 1], F32)
            nc.vector.tensor_tensor(ssd[:], SSall, SqS, op=Alu.subtract)
            nc.vector.tensor_tensor(ssd[:], ssd[:], beta[:], op=Alu.mult)
            qa = sb.tile([16, 1], F32)
            nc.vector.scalar_tensor_tensor(qa[:], SqS, alpha[:], ssd[:],
                                           op0=Alu.mult, op1=Alu.subtract)
            nc.vector.tensor_scalar(raug[:, D + 1:D + 2], qa[:], MARGIN, None,
                                    op0=Alu.add)

            # fold validity into the coefficients: zero Raug rows of classes
            # with cnt < 2, so relu(loss) vanishes for invalid rows
            vmask = sb.tile([16, 1], F32)
            nc.vector.tensor_scalar(vmask[:], cnt, 1.5, None, op0=Alu.is_ge)
            nc.vector.tensor_scalar(raug[:], raug[:], vmask[:], None, op0=Alu.mult)

            rhi = sb.tile([16, D + 2], BF16)
            nc.vector.tensor_copy(rhi[:], raug[:])

            # ---------------- per-row losses ----------------
            lossrows = sb.tile([128, T], F32)
            for t in range(T):
                dP = ps.tile([128, 512], F32, tag="dpsum", bufs=4, name=f"dP{t}")
                nc.tensor.matmul(dP[:, 0:D + 2], eohT[:, t * 128:(t + 1) * 128],
                                 rhi[:], start=True, stop=True)
                pscr = sb.tile([128, D + 2], BF16, tag="pscr", bufs=4,
                               name=f"pt{t}")
                nc.vector.scalar_tensor_tensor(
                    pscr[:], dP[:, 0:D + 2], 0.0, fa3[:, t, 0:D + 2],
                    op0=Alu.bypass, op1=Alu.mult,
                    accum_out=lossrows[:, t:t + 1])

            # ---------------- final reduction ----------------
            accpair = sb.tile([128, 2], F32)
            nc.vector.tensor_copy(accpair[:, 1:2], csts[:, 17:18])  # zeros col
            relscr = sb.tile([128, T], F32)
            nc.vector.tensor_scalar(relscr[:], lossrows[:], 0.0, None,
                                    op0=Alu.max, op1=Alu.add,
                                    accum_out=accpair[:, 0:1])
            nc.vector.tensor_tensor(accpair[0:16, 1:2], cnt, vmask[:],
                                    op=Alu.mult)
            finP = ps.tile([1, 2], F32)
            nc.tensor.matmul(finP[:], csts[:, 0:1], accpair[:],
                             start=True, stop=True)
            fin = sb.tile([1, 2], F32)
            nc.vector.tensor_copy(fin[:], finP[:])
            den = sb.tile([1, 1], F32)
            nc.vector.tensor_scalar(den[:], fin[:, 1:2], 1.0, None, op0=Alu.max)
            nc.vector.reciprocal(den[:], den[:])
            resS = sb.tile([1, 1], F32)
            nc.vector.tensor_tensor(resS[:], fin[:, 0:1], den[:], op=Alu.mult)
            nc.sync.dma_start(res, resS[:])

    nc.compile()
    _CACHE["nc"] = nc
    return nc


def _make_in_maps(features, labels):
    feats = np.ascontiguousarray(np.asarray(features, dtype=np.float32))
    lab = np.ascontiguousarray(np.asarray(labels)).astype(np.int64)

    cst = np.zeros((128, CW), np.float32)
    cst[:, 0] = 1.0
    cst[0:16, 1:17] = 1.0

    fa = np.zeros((128, T, W), np.float32)
    fa[:, :, 0:D] = feats.reshape(T, 128, D).transpose(1, 0, 2)
    fa[:, :, D + 1] = 1.0
    fb = fa.reshape(128, T * W).astype(ml_dtypes.bfloat16)

    labT = lab.reshape(T, 128).T                       # (128, T)
    eoh = (labT[:, :, None] == np.arange(16)[None, None, :])
    eohin = np.ascontiguousarray(
        eoh.reshape(128, T * 16)).astype(ml_dtypes.bfloat16)
    eohT = (lab[None, :] == np.arange(16)[:, None])
    eohTin = np.ascontiguousarray(eohT).astype(ml_dtypes.bfloat16)

    one = {
        "fbin": fb,
        "eohin": eohin,
        "eohTin": eohTin,
        "cst": cst,
    }
    return [dict(one) for _ in range(NCORES)]


def kernel(features, labels):
    nc = _build()
    in_maps = _make_in_maps(features, labels)
    out = run_bass_kernel_spmd(nc, in_maps, core_ids=list(range(NCORES)))
    return np.float32(out.results[0]["res"][0, 0])


# revision 15
# speedup vs baseline: 1.1393x; 1.0369x over previous
"""Trainium2 Bass kernel for nn_ContrastiveDist (supervised contrastive loss).

Math
----
The reference builds (n,n) distance/weight matrices, but the loss collapses
exactly to per-class statistics.  With classes c = 0..15, per-class count
cnt[c], feature sums C[c,:], squared-norm sums SqSum[c], global sums
Ftot / SSall:

    alpha[c] = 1/(cnt[c]-1+eps)
    beta[c]  = 1/(n-cnt[c]+eps)
    loss_i   = sq_i*P[c_i] + (Q[c_i]+M) + f_i . R[c_i]
      P[c]   = alpha*cnt - beta*(n-cnt)
      Q[c]   = alpha*SqSum[c] - beta*(SSall-SqSum[c])
      R[c,:] = 2*beta*(Ftot-C[c]) - 2*alpha*C[c]
    result   = sum(relu(loss_i)*valid_i) / max(sum(valid_i), 1)

valid_i = (cnt[c_i] >= 2) is folded into the coefficients: R/P/Q rows of
invalid classes are zeroed, so relu(loss) = 0 there, and the valid count
comes from sum(cnt[c]*vmask[c]).

Precision: feature path is single-chain bf16 with fp32 PSUM accumulation;
numpy pipeline sim puts the result at rel err <= ~1e-3 (harness gate 2e-2).

Perf structure (per core, inputs replicated -- collectives measured at
~45us/call under this dispatch path, so none are used):
  - one contiguous bf16 copy of F (fmain) + a tiny [sq,1] side tensor;
    one-hots precomputed on host
  - sq: chunked Square (split Scalar/DVE) + DVE reduce, pipelined with load
  - stats: two accumulating matmuls per tile (C from fmain, [SqS,cnt] from
    the side tensor), chunk-pipelined behind the load
  - F is transposed on-device with XBAR dma_start_transpose during the
    otherwise-idle DMA window, so the per-row loss becomes G_t = F_t @ R^T
    (constant stationary weights, N=16 streams) instead of 64 serial
     one-hot gathers + 204ns DVE dot products
  - per-row pick: chunked (G3 * onehot) multiply + innermost reduce on DVE;
    the sq*P + QM terms come from one broadcasted pick over (128,T,2,16)
"""

import numpy as np
import ml_dtypes

import concourse.bacc as bacc
import concourse.tile as tile
import concourse.mybir as mybir
from concourse.bass_utils import run_bass_kernel_spmd

N, D, K, NCORES = 8192, 128, 16, 8
T = N // 128               # 64 row-tiles of 128
EPS, MARGIN = 1e-6, 10.0
F32 = mybir.dt.float32
BF16 = mybir.dt.bfloat16
Alu = mybir.AluOpType
Act = mybir.ActivationFunctionType
AxX = mybir.AxisListType.X

# const tensor (128, CW) f32:
#   col 0        1.0 everywhere (ones(128,1) lhsT for the final reduce)
#   cols 1:17    1.0 in rows 0:16 (ones(16,16) lhsT for global sums)
#   col 17       0.0 (zeros column)
#   cols 18:34   I16 in rows 0:16 (identity for the PE transpose of R)
#   cols 34:162  1.0 in rows 0:16 (ones(16,128) lhsT for P/QM broadcast)
CW = 162

_CACHE: dict = {}


def _build():
    if "nc" in _CACHE:
        return _CACHE["nc"]

    nc = bacc.Bacc("TRN2", target_bir_lowering=False, debug=False, num_devices=NCORES)
    fmin_ = nc.dram_tensor("fmain", [128, T * D], BF16, kind="ExternalInput").ap()
    fsqin = nc.dram_tensor("fsqin", [128, T * 2], BF16, kind="ExternalInput").ap()
    eohin = nc.dram_tensor("eohin", [128, T * 16], BF16, kind="ExternalInput").ap()
    cst = nc.dram_tensor("cst", [128, CW], F32, kind="ExternalInput").ap()
    res = nc.dram_tensor("res", [1, 1], F32, kind="ExternalOutput").ap()

    with tile.TileContext(nc) as tc:
        with (
            tc.tile_pool(name="sb", bufs=1) as sb,
            tc.tile_pool(name="ps", bufs=1, space="PSUM") as ps,
        ):
            # ---------------- loads ----------------
            csts = sb.tile([128, CW], F32)
            nc.gpsimd.dma_start(csts[:], cst)
            eoh = sb.tile([128, T * 16], BF16)
            nc.gpsimd.dma_start(eoh[:], eohin)
            fsq1 = sb.tile([128, T * 2], BF16)
            nc.gpsimd.dma_start(fsq1[:], fsqin)

            fmain = sb.tile([128, T * D], BF16)
            fm3 = fmain.rearrange("p (t d) -> p t d", d=D)
            fs3 = fsq1.rearrange("p (t x) -> p t x", x=2)
            # (start_tile, n_tiles, square_engine)
            chunks = [(0, 8, "act"), (8, 12, "dve"), (20, 12, "act"),
                      (32, 12, "dve"), (44, 8, "act"), (52, 6, "dve"),
                      (58, 6, "act")]
            qengs = [nc.sync, nc.scalar]
            for g, (t0, ntl, _) in enumerate(chunks):
                eng = qengs[g % len(qengs)]
                eng.dma_start(fmain[:, t0 * D:(t0 + ntl) * D],
                              fmin_[:, t0 * D:(t0 + ntl) * D])

            # on-device transpose of F (XBAR), 16 tiles per call, overlapped
            # with the sq/stats pipeline in the otherwise-idle DMA window
            faT = sb.tile([128, T * D], BF16)
            faT3 = faT.rearrange("p (t r) -> p t r", r=128)
            for x in range(4):
                t0 = x * 16
                eng = qengs[x % 2]
                eng.dma_start_transpose(
                    faT3[:, t0:t0 + 16, :],
                    fmain[:, t0 * D:(t0 + 16) * D])

            eoh3 = eoh.rearrange("p (t c) -> p t c", c=16)

            # ---------- sq + per-class stats, chunk-pipelined with load ----
            sqd = sb.tile([128, T], F32)
            statsP = ps.tile([16, D], F32)
            statsP2 = ps.tile([16, 2], F32)
            for g, (t0, ntl, sqeng) in enumerate(chunks):
                scr = sb.tile([128, ntl * D], BF16, tag="sqscr", bufs=2,
                              name=f"scr{g}")
                scr3 = scr.rearrange("p (t d) -> p t d", d=D)
                if sqeng == "act":
                    nc.scalar.activation(scr3[:, :, :],
                                         fm3[:, t0:t0 + ntl, :], Act.Square)
                else:
                    nc.vector.tensor_tensor(scr3[:, :, :],
                                            fm3[:, t0:t0 + ntl, :],
                                            fm3[:, t0:t0 + ntl, :],
                                            op=Alu.mult)
                nc.vector.tensor_reduce(sqd[:, t0:t0 + ntl], scr3,
                                        axis=AxX, op=Alu.add)
                nc.vector.tensor_copy(fs3[:, t0:t0 + ntl, 0],
                                      sqd[:, t0:t0 + ntl])
                for j in range(ntl):
                    t = t0 + j
                    nc.tensor.matmul(statsP[:], eoh3[:, t, :],
                                     fm3[:, t, :],
                                     start=(t == 0), stop=(t == T - 1))
                    nc.tensor.matmul(statsP2[:], eoh3[:, t, :],
                                     fs3[:, t, :],
                                     start=(t == 0), stop=(t == T - 1))
            stats = sb.tile([16, D + 2], F32)
            nc.vector.tensor_copy(stats[:, 0:D], statsP[:])
            nc.vector.tensor_copy(stats[:, D:D + 2], statsP2[:])

            # ---------------- per-class coefficients ----------------
            C = stats[:, 0:D]
            SqS = stats[:, D:D + 1]
            cnt = stats[:, D + 1:D + 2]
            gbP = ps.tile([16, D + 2], F32)
            nc.tensor.matmul(gbP[:], csts[0:16, 1:17], stats[:],
                             start=True, stop=True)
            gb = sb.tile([16, D + 2], F32)
            nc.vector.tensor_copy(gb[:], gbP[:])
            Ftot = gb[:, 0:D]
            SSall = gb[:, D:D + 1]

            alpha = sb.tile([16, 1], F32)
            nc.vector.tensor_scalar(alpha[:], cnt, EPS - 1.0, None, op0=Alu.add)
            nc.vector.reciprocal(alpha[:], alpha[:])
            beta = sb.tile([16, 1], F32)
            nc.vector.tensor_scalar(beta[:], cnt, -1.0, float(N) + EPS,
                                    op0=Alu.mult, op1=Alu.add)
            nc.vector.reciprocal(beta[:], beta[:])
            nalpha2 = sb.tile([16, 1], F32)
            nc.vector.tensor_scalar(nalpha2[:], alpha[:], -2.0, None, op0=Alu.mult)
            beta2 = sb.tile([16, 1], F32)
            nc.vector.tensor_scalar(beta2[:], beta[:], 2.0, None, op0=Alu.mult)

            raug = sb.tile([16, D + 2], F32)
            tmpd = sb.tile([16, D], F32)
            nc.vector.tensor_tensor(tmpd[:], Ftot, C, op=Alu.subtract)
            nc.vector.tensor_scalar(tmpd[:], tmpd[:], beta2[:], None, op0=Alu.mult)
            nc.vector.scalar_tensor_tensor(raug[:, 0:D], C, nalpha2[:], tmpd[:],
                                           op0=Alu.mult, op1=Alu.add)
            nmc = sb.tile([16, 1], F32)
            nc.vector.tensor_scalar(nmc[:], cnt, -1.0, float(N),
                                    op0=Alu.mult, op1=Alu.add)
            nc.vector.tensor_tensor(nmc[:], nmc[:], beta[:], op=Alu.mult)
            nc.vector.scalar_tensor_tensor(raug[:, D:D + 1], cnt, alpha[:], nmc[:],
                                           op0=Alu.mult, op1=Alu.subtract)
            ssd = sb.tile([16, 1], F32)
            nc.vector.tensor_tensor(ssd[:], SSall, SqS, op=Alu.subtract)
            nc.vector.tensor_tensor(ssd[:], ssd[:], beta[:], op=Alu.mult)
            qa = sb.tile([16, 1], F32)
            nc.vector.scalar_tensor_tensor(qa[:], SqS, alpha[:], ssd[:],
                                           op0=Alu.mult, op1=Alu.subtract)
            nc.vector.tensor_scalar(raug[:, D + 1:D + 2], qa[:], MARGIN, None,
                                    op0=Alu.add)

            # fold validity: zero rows of classes with cnt < 2
            vmask = sb.tile([16, 1], F32)
            nc.vector.tensor_scalar(vmask[:], cnt, 1.5, None, op0=Alu.is_ge)
            nc.vector.tensor_scalar(raug[:], raug[:], vmask[:], None, op0=Alu.mult)

            # R^T (128, 16) via PE transpose; P/QM broadcast to all partitions
            rtP = ps.tile([128, 16], F32)
            nc.tensor.transpose(rtP[:], raug[:, 0:D], csts[0:16, 18:34])
            rtb = sb.tile([128, 16], BF16)
            nc.vector.tensor_copy(rtb[:], rtP[:])

            pqdiag = sb.tile([16, 32], F32)
            nc.vector.tensor_scalar(pqdiag[:, 0:16], csts[0:16, 18:34],
                                    raug[:, D:D + 1], None, op0=Alu.mult)
            nc.vector.tensor_scalar(pqdiag[:, 16:32], csts[0:16, 18:34],
                                    raug[:, D + 1:D + 2], None, op0=Alu.mult)
            pqbP = ps.tile([128, 32], F32)
            nc.tensor.matmul(pqbP[:], csts[0:16, 34:162], pqdiag[:],
                             start=True, stop=True)
            pqb = sb.tile([128, 32], BF16)
            nc.vector.tensor_copy(pqb[:], pqbP[:])

            # ---------------- per-row losses ----------------
            # PQrow[p,t,0] = P[label], PQrow[p,t,1] = QM[label]
            eohdup = eoh3.unsqueeze(2).broadcast_to((128, T, 2, 16))
            pq3 = pqb.rearrange("p (x c) -> p x c", c=16)
            pqb4 = pq3.unsqueeze(1).broadcast_to((128, T, 2, 16))
            pqscr = sb.tile([128, T * 2 * 16], BF16)
            pqscr4 = pqscr.rearrange("p (t x c) -> p t x c", x=2, c=16)
            nc.vector.tensor_tensor(pqscr4[:, :, :, :], eohdup, pqb4,
                                    op=Alu.mult)
            pqrow = sb.tile([128, T * 2], F32)
            pqrow3 = pqrow.rearrange("p (t x) -> p t x", x=2)
            nc.vector.tensor_reduce(pqrow3[:, :, :], pqscr4, axis=AxX,
                                    op=Alu.add)

            # G3 chunks: G[p,t,c] = f_(t,p) . R[c]
            GCH = 8
            gpick = sb.tile([128, T], F32)
            for g in range(T // GCH):
                t0 = g * GCH
                gP = ps.tile([128, GCH * 16], F32, tag="gpsum", bufs=2,
                             name=f"gP{g}")
                gP3 = gP.rearrange("p (t c) -> p t c", c=16)
                for j in range(GCH):
                    nc.tensor.matmul(gP[:, j * 16:(j + 1) * 16],
                                     faT3[:, t0 + j, :], rtb[:],
                                     start=True, stop=True)
                pick = sb.tile([128, GCH * 16], F32, tag="pick", bufs=2,
                               name=f"pick{g}")
                pick3 = pick.rearrange("p (t c) -> p t c", c=16)
                nc.vector.tensor_tensor(pick3[:, :, :], gP3,
                                        eoh3[:, t0:t0 + GCH, :], op=Alu.mult)
                nc.vector.tensor_reduce(gpick[:, t0:t0 + GCH], pick3,
                                        axis=AxX, op=Alu.add)

            # lossrows = gpick + sq*Prow + QMrow
            lossrows = sb.tile([128, T], F32)
            nc.vector.scalar_tensor_tensor(
                lossrows[:], sqd[:], 0.0, pqrow3[:, :, 0].squeeze(),
                op0=Alu.bypass, op1=Alu.mult)
            nc.vector.tensor_tensor(lossrows[:], lossrows[:],
                                    pqrow3[:, :, 1].squeeze(), op=Alu.add)
            nc.vector.tensor_tensor(lossrows[:], lossrows[:], gpick[:],
                                    op=Alu.add)

            # ---------------- final reduction ----------------
            accpair = sb.tile([128, 2], F32)
            nc.vector.tensor_copy(accpair[:, 1:2], csts[:, 17:18])  # zeros
            relscr = sb.tile([128, T], F32)
            nc.vector.tensor_scalar(relscr[:], lossrows[:], 0.0, None,
                                    op0=Alu.max, op1=Alu.add,
                                    accum_out=accpair[:, 0:1])
            nc.vector.tensor_tensor(accpair[0:16, 1:2], cnt, vmask[:],
                                    op=Alu.mult)
            finP = ps.tile([1, 2], F32)
            nc.tensor.matmul(finP[:], csts[:, 0:1], accpair[:],
                             start=True, stop=True)
            fin = sb.tile([1, 2], F32)
            nc.vector.tensor_copy(fin[:], finP[:])
            den = sb.tile([1, 1], F32)
            nc.vector.tensor_scalar(den[:], fin[:, 1:2], 1.0, None, op0=Alu.max)
            nc.vector.reciprocal(den[:], den[:])
            resS = sb.tile([1, 1], F32)
            nc.vector.tensor_tensor(resS[:], fin[:, 0:1], den[:], op=Alu.mult)
            nc.sync.dma_start(res, resS[:])

    nc.compile()
    _CACHE["nc"] = nc
    return nc


def _make_in_maps(features, labels):
    feats = np.ascontiguousarray(np.asarray(features, dtype=np.float32))
    lab = np.ascontiguousarray(np.asarray(labels)).astype(np.int64)

    cst = np.zeros((128, CW), np.float32)
    cst[:, 0] = 1.0
    cst[0:16, 1:17] = 1.0
    cst[0:16, 18:34] = np.eye(16, dtype=np.float32)
    cst[0:16, 34:162] = 1.0

    fmain = feats.reshape(T, 128, D).transpose(1, 0, 2).reshape(128, T * D)
    fmain = fmain.astype(ml_dtypes.bfloat16)
    fsq = np.zeros((128, T, 2), np.float32)
    fsq[:, :, 1] = 1.0
    fsq = fsq.reshape(128, T * 2).astype(ml_dtypes.bfloat16)

    labT = lab.reshape(T, 128).T                       # (128, T)
    eoh = (labT[:, :, None] == np.arange(16)[None, None, :])
    eohin = np.ascontiguousarray(
        eoh.reshape(128, T * 16)).astype(ml_dtypes.bfloat16)

    one = {
        "fmain": fmain,
        "fsqin": fsq,
        "eohin": eohin,
        "cst": cst,
    }
    return [dict(one) for _ in range(NCORES)]


def kernel(features, labels):
    nc = _build()
    in_maps = _make_in_maps(features, labels)
    out = run_bass_kernel_spmd(nc, in_maps, core_ids=list(range(NCORES)))
    return np.float32(out.results[0]["res"][0, 0])
